# revision 1
# baseline (speedup 1.0000x reference)
"""BarrierNet (MLP heads + dCBF closed-form QP) Trainium2 Bass kernel.

Data-parallel over 8 NeuronCores: batch 262144 is split into 8 shards of
32768 rows; the tiny MLP weights are replicated (folded with mean/std on
host) and each core computes its full shard independently. No collectives.

Per-core dataflow (feature-major matmuls, batch in the free dim):
  L1: h^T = relu(W1eff @ x^T + b1eff)      [128, 512] per 512-row chunk
      (4x row-group packed via tile_position: 4 chunks run concurrently
       in disjoint 32-row groups of the PE array since K=8)
  L2: mid^T = relu(Wmid @ h^T + bmid)      Wmid = vstack(W21, W22)
  L3: head accumulation trick: chunk j uses a sliding window of a
      zero-padded weight tensor so its 4 head outputs land on PSUM
      partitions {32v + j}; 32 chunks accumulate into one [128, 512]
      PSUM tile -> full-width copies out of PSUM instead of 4-row ones.
  QP: per half-core (16384 rows), heads are reshaped (SBUF->SBUF DMA)
      to batch-major [128, 128] tiles (row r_half = p*128 + c) and the
      dCBF/QP elementwise math runs at full 128-partition utilization
      on VectorE/ScalarE, overlapping the other half's matmuls.

Matmuls run in float32r (TF32-like on the PE; ~1e-4 end-to-end scaled
error measured) at 2 cycles/col vs fp32's 4. A dummy-matmul warmup burst
keeps the PE HAM clock at 2.4 GHz through the input-DMA phase.
"""

import os
import sys

import numpy as np

sys.path.insert(0, "/opt/trn_rl_repo")

import concourse.bass as bass
import concourse.tile as tile
from concourse import mybir
from concourse.bass_utils import run_bass_kernel_spmd

F32 = mybir.dt.float32
F32R = mybir.dt.float32r
BF16 = mybir.dt.bfloat16
AF = mybir.ActivationFunctionType
ALU = mybir.AluOpType

B = 262144
NF = 8
NCORES = 8
BC = B // NCORES   # 32768 rows per core
CH = 512           # chunk columns (one PSUM bank of fp32)
NGRP = 4           # L1 row groups (tile_position packing)
GSZ = BC // NGRP   # 8192 rows per group
NSLOT = GSZ // CH  # 16 chunks per group
HB = BC // 2       # 16384 rows per half
HQ = HB // 128     # 128: per-half batch-major free width
XSLICE = 4         # xt load slices per group
OBS_X, OBS_Y, RAD = 4.0, 6.0, 1.5
PI = float(np.pi)

# engine split of the mid-relu copies: every Nth goes to ScalarE
M_COPY_ACT_EVERY = 5
N_WARMUP_MM = 30

_MMDT_MAP = {"f32r": F32R, "f32": F32, "bf16": BF16}
MMDT = _MMDT_MAP[os.environ.get("KERNEL_MM_DTYPE", "f32r")]
MMNP = {"f32r": np.float32, "f32": np.float32}.get(
    os.environ.get("KERNEL_MM_DTYPE", "f32r"))  # None -> ml_dtypes.bfloat16


def _build_program(std4, mean4, split_waits=True, reps=1):
    nc = bass.Bass()

    x_bm = nc.dram_tensor("x_bm", [BC, NF], F32, kind="ExternalInput")
    x_tr = nc.dram_tensor("x_tr", [32, GSZ], MMDT, kind="ExternalInput")
    w1t = nc.dram_tensor("w1t", [8, 128], MMDT, kind="ExternalInput")
    wmw = nc.dram_tensor("wmw", [128, 288], MMDT, kind="ExternalInput")
    bias3 = nc.dram_tensor("bias3", [128, 3], F32, kind="ExternalInput")
    u_out = nc.dram_tensor("u", [BC, 2], F32, kind="ExternalOutput")

    with tile.TileContext(nc) as tc:
        from contextlib import ExitStack

        with ExitStack() as ctx:
            _body(ctx, tc, x_bm, x_tr, w1t, wmw, bias3, u_out,
                  std4, mean4, reps)
    if split_waits:
        _split_multi_waits(nc)
    return nc


def _split_multi_waits(nc):
    """walrus (this build) accepts at most one sync-wait per instruction;
    merge same-semaphore waits to their max threshold, then hoist any
    remaining extra waits onto standalone same-engine EventSemaphore ops."""
    for blk in nc.main_func.blocks:
        out = []
        for ins in blk.instructions:
            si = ins.sync_info
            waits = list(si.on_wait) if si is not None else []
            if len(waits) > 1:
                merged = {}
                for w in waits:
                    key = (w.sync_type, w.id)
                    prev = merged.get(key)
                    if (prev is None or (w.wait_value or 0) >
                            (prev.wait_value or 0)):
                        merged[key] = w
                waits = list(merged.values())
                if len(waits) == 1:
                    ins.sync_info = type(si)(on_wait=waits,
                                             on_update=list(si.on_update))
            if len(waits) > 1:
                for k, w in enumerate(waits[:-1]):
                    ev = mybir.InstEventSemaphore(
                        name=f"{ins.name}w{k}", ins=[], outs=[])
                    ev.engine = ins.engine
                    ev.sync_info = type(si)(on_wait=[w], on_update=[])
                    out.append(ev)
                ins.sync_info = type(si)(on_wait=[waits[-1]],
                                         on_update=list(si.on_update))
            out.append(ins)
        blk.instructions = out


def _body(ctx, tc, x_bm, x_tr, w1t, wmw, bias3, u_out,
          std4, mean4, reps):
    nc = tc.nc

    const = ctx.enter_context(tc.tile_pool(name="const", bufs=1))
    xtp = ctx.enter_context(tc.tile_pool(name="xtp", bufs=1))
    hp = ctx.enter_context(tc.tile_pool(name="hp", bufs=3))
    mp = ctx.enter_context(tc.tile_pool(name="mp", bufs=3))
    hs = ctx.enter_context(tc.tile_pool(name="hs", bufs=1))
    qp = ctx.enter_context(tc.tile_pool(name="qp", bufs=1))
    # PSUM: h pairs [128,1024] x2 bufs = 4 banks; m [128,512] x2 = 2;
    # two head accumulators = 2  -> exactly 8 banks
    ps_h = ctx.enter_context(tc.tile_pool(name="ps_h", bufs=2, space="PSUM"))
    ps_m = ctx.enter_context(tc.tile_pool(name="ps_m", bufs=2, space="PSUM"))
    ps_hd = ctx.enter_context(tc.tile_pool(name="ps_hd", bufs=1, space="PSUM"))

    # ---- constants / weights to SBUF: 3 DMAs, w1g first (gates L1) ----
    w1g_sb = const.tile([128, 128], MMDT)   # W1eff^T in 4 row groups
    for g in range(NGRP):
        eng = nc.sync if g % 2 == 0 else nc.scalar
        eng.dma_start(out=w1g_sb[32 * g:32 * g + 8, :], in_=w1t[:, :])
    wmw_sb = const.tile([128, 288], MMDT)
    nc.sync.dma_start(out=wmw_sb, in_=wmw[:, :])
    wmt_sb = wmw_sb[:, 0:128]
    wz_sb = wmw_sb[:, 128:288]
    bias3_sb = const.tile([128, 3], F32)
    nc.scalar.dma_start(out=bias3_sb, in_=bias3[:, :])
    b1_sb = bias3_sb[:, 0:1]
    bm_sb = bias3_sb[:, 1:2]
    bh_sb = bias3_sb[:, 2:3]

    for _ in range(reps):
        _body_rep(nc, tc, const, xtp, hp, mp, hs, qp, ps_h, ps_m, ps_hd,
                  x_bm, x_tr, u_out, w1g_sb, wmt_sb, wz_sb, b1_sb, bm_sb,
                  bh_sb, std4, mean4)


def _qp_pre(nc, qp, half, x_sb, std4, mean4):
    """x-only dCBF terms for one half (r_half = p*128 + c). Runs during
    the matmul phase. Returns the tile dict for _qp_post."""
    s0, s1c, s2c, s3 = std4
    m0, m1c, m2c, m3 = mean4

    def t(name):
        nm = f"{name}_{half}"
        return qp.tile([128, HQ], F32, name=nm, tag=nm)

    xs3 = x_sb[:].rearrange("p (c f) -> p c f", f=NF)
    X0, X1, X2, X3 = (xs3[:, :, i] for i in range(4))

    ST, CT, DX, DY, V = t("ST"), t("CT"), t("DX"), t("DY"), t("V")

    def wrapped_sin(out, phase_bias, nm):
        ph = t(f"ph{nm}")
        msk = t(f"mk{nm}")
        nc.vector.tensor_scalar(ph, X2, s2c, phase_bias, ALU.mult, ALU.add)
        nc.vector.tensor_scalar(msk, ph, PI, None, ALU.is_gt)
        nc.vector.scalar_tensor_tensor(ph, msk, -2.0 * PI, ph,
                                       ALU.mult, ALU.add)
        nc.vector.tensor_scalar(msk, ph, -PI, None, ALU.is_lt)
        nc.vector.scalar_tensor_tensor(ph, msk, 2.0 * PI, ph,
                                       ALU.mult, ALU.add)
        nc.scalar.activation(out, ph, AF.Sin)

    wrapped_sin(ST, m2c, "s")
    wrapped_sin(CT, m2c + PI / 2, "c")
    nc.vector.tensor_scalar(DX, X0, s0, m0 - OBS_X, ALU.mult, ALU.add)
    nc.vector.tensor_scalar(DY, X1, s1c, m1c - OBS_Y, ALU.mult, ALU.add)
    nc.vector.tensor_scalar(V, X3, s3, m3, ALU.mult, ALU.add)

    t1, t2, Aq, t3, t4, Bq = (t("t1"), t("t2"), t("Aq"), t("t3"), t("t4"),
                              t("Bq"))
    nc.vector.tensor_tensor(t1, DX, CT, ALU.mult)
    nc.vector.tensor_tensor(t2, DY, ST, ALU.mult)
    nc.vector.tensor_tensor(Aq, t1, t2, ALU.add)       # A = dx ct + dy st
    nc.vector.tensor_tensor(t3, DX, ST, ALU.mult)
    nc.vector.tensor_tensor(t4, DY, CT, ALU.mult)
    nc.vector.tensor_tensor(Bq, t3, t4, ALU.subtract)  # B = dx st - dy ct

    VB, VA = t("VB"), t("VA")
    nc.vector.tensor_tensor(VB, V, Bq, ALU.mult)       # G1 = 2 VB
    nc.vector.tensor_tensor(VA, V, Aq, ALU.mult)       # bdot = 2 VA

    DX2, DY2, BARp, V2d, VB2, A2 = (t("DX2"), t("DY2"), t("BARp"),
                                    t("V2d"), t("VB2"), t("A2"))
    nc.scalar.activation(DX2, DX, AF.Square)
    nc.scalar.activation(DY2, DY, AF.Square)
    nc.vector.tensor_tensor(BARp, DX2, DY2, ALU.add)   # dx^2 + dy^2
    nc.scalar.activation(V2d, V, AF.Square, scale=float(np.sqrt(2.0)))
    nc.scalar.activation(VB2, VB, AF.Square, scale=2.0)  # G1^2
    nc.scalar.activation(A2, Aq, AF.Square, scale=2.0)   # G2^2

    GG, R = t("GG"), t("R")
    nc.vector.scalar_tensor_tensor(GG, VB2, 1e-12, A2, ALU.add, ALU.add)
    nc.vector.reciprocal(R, GG)
    return dict(Aq=Aq, VB=VB, VA=VA, BARp=BARp, V2d=V2d, R=R, t=t)


def _qp_post(nc, qp, half, pre, headsb, u_out, ve):
    """Head-dependent QP tail for one half."""
    t = pre["t"]
    Aq, VB, VA = pre["Aq"], pre["VB"], pre["VA"]
    BARp, V2d, R = pre["BARp"], pre["V2d"], pre["R"]

    p1n, p2n, sg1, sg2 = t("p1n"), t("p2n"), t("sg1"), t("sg2")
    for v, dst in enumerate([p1n, p2n, sg1, sg2]):
        eng = nc.sync if v % 2 == 0 else nc.gpsimd
        eng.dma_start(
            out=dst,
            in_=headsb[32 * v:32 * v + 32, :].rearrange(
                "j (q c) -> j q c", q=4),
        )

    SS, SP, T5p, T4d = t("SS"), t("SP"), t("T5p"), t("T4d")
    ve.tensor_tensor(SS, sg1, sg2, ALU.add)
    ve.tensor_tensor(SP, sg1, sg2, ALU.mult)
    ve.scalar_tensor_tensor(T5p, BARp, -RAD * RAD, SP,
                                   ALU.add, ALU.mult)
    ve.scalar_tensor_tensor(T4d, SS, 8.0, VA, ALU.mult, ALU.mult)

    T1d, T2d, T3d, q1, q2, NUMn = (t("T1d"), t("T2d"), t("T3d"),
                                   t("q1"), t("q2"), t("NUMn"))
    ve.scalar_tensor_tensor(T1d, VB, 2.0, p1n, ALU.mult, ALU.mult)
    ve.scalar_tensor_tensor(T2d, Aq, 2.0, p2n, ALU.mult, ALU.mult)
    ve.tensor_tensor(T3d, T1d, T2d, ALU.subtract)  # = -Gp
    ve.tensor_tensor(q1, T3d, V2d, ALU.subtract)
    ve.tensor_tensor(q2, q1, T4d, ALU.subtract)
    ve.scalar_tensor_tensor(NUMn, T5p, 16.0, q2,
                                   ALU.mult, ALU.subtract)  # = Gp + hcon

    L0, LAM2 = t("L0"), t("LAM2")
    ve.tensor_tensor(L0, NUMn, R, ALU.mult)
    ve.tensor_scalar(LAM2, L0, -2.0, 0.0, ALU.mult, ALU.max)  # 2 lam

    u_bm = qp.tile([128, 2 * HQ], F32, name=f"u_bm_{half}",
                   tag=f"u_bm_{half}")
    ub3 = u_bm[:].rearrange("p (c v) -> p c v", v=2)
    m1t, m2t = t("m1t"), t("m2t")
    ve.tensor_tensor(m1t, LAM2, VB, ALU.mult)
    ve.tensor_tensor(ub3[:, :, 0], p1n, m1t, ALU.subtract)
    ve.tensor_tensor(m2t, LAM2, Aq, ALU.mult)
    ve.tensor_tensor(ub3[:, :, 1], p2n, m2t, ALU.add)

    nc.gpsimd.dma_start(
        out=u_out[half * HB:(half + 1) * HB, :].rearrange(
            "(p c) v -> p c v", p=128),
        in_=ub3,
    )


def _body_rep(nc, tc, const, xtp, hp, mp, hs, qp, ps_h, ps_m, ps_hd,
              x_bm, x_tr, u_out, w1g_sb, wmt_sb, wz_sb, b1_sb, bm_sb, bh_sb,
              std4, mean4):
    # ---- head accumulators; also the PE-warmup dump target ----
    head_ps = [ps_hd.tile([128, CH], F32, name=f"head{h}", tag=f"head{h}")
               for h in range(2)]

    # PE warmup: dummy matmuls into head_ps[0] (overwritten by the real
    # accumulation's start=True later) keep the HAM clock warm while the
    # input DMAs run.
    for w in range(N_WARMUP_MM):
        nc.tensor.matmul(head_ps[0][:, 0:128], w1g_sb[0:8, :],
                         w1g_sb[0:8, 0:128], start=True, stop=True)

    # ---- x loads, sliced so compute starts after the first slice ----
    xt_sb = xtp.tile([128, GSZ], MMDT, name="xt_sb", tag="xt_sb")
    SL = NSLOT // XSLICE * CH  # columns per slice
    i = 0
    for half in range(2):
        for s in range(XSLICE):
            for g in (2 * half, 2 * half + 1):
                eng = nc.gpsimd if i % 2 == 0 else nc.sync
                i += 1
                eng.dma_start(
                    out=xt_sb[32 * g:32 * g + 8, s * SL:(s + 1) * SL],
                    in_=x_tr[8 * g:8 * g + 8, s * SL:(s + 1) * SL])
    # batch-major x per half for the dCBF math: r_half = p*128 + c
    x_half = []
    for h in range(2):
        xh = xtp.tile([128, HQ * NF], F32, name=f"x_sb{h}", tag=f"x_sb{h}")
        x_half.append(xh)
        nc.scalar.dma_start(
            out=xh[:].rearrange("p (c f) -> p c f", f=NF),
            in_=x_bm[h * HB:(h + 1) * HB, :].rearrange(
                "(p c) f -> p c f", p=128),
        )

    qp_pre = [None, None]

    # ---- MLP chunk pipeline: half 0 fully, then half 1 ----
    mcopy_i = 0
    for half in range(2):
        for slot in range(NSLOT):
            # L1 for this half's two groups, 2-way row-group packed,
            # written into one [128, 1024] PSUM pair
            h_ps = ps_h.tile([128, 2 * CH], F32, name="h_ps", tag="h_ps")
            for k in range(2):
                g = 2 * half + k
                nc.tensor.matmul(
                    h_ps[:, k * CH:(k + 1) * CH],
                    w1g_sb[32 * g:32 * g + 8, :],
                    xt_sb[32 * g:32 * g + 8, slot * CH:(slot + 1) * CH],
                    start=True, stop=True,
                    tile_position=(32 * g, 0),
                )
            h_sb = hp.tile([128, 2 * CH], MMDT, name="h_sb", tag="h_sb")
            nc.scalar.activation(h_sb, h_ps, AF.Relu, bias=b1_sb, scale=1.0)

            for k in range(2):
                jh = k * NSLOT + slot         # head slot within half
                step = slot * 2 + k           # accumulation step

                m_ps = ps_m.tile([128, CH], F32, name="m_ps", tag="m_ps")
                nc.tensor.matmul(
                    m_ps, wmt_sb, h_sb[:, k * CH:(k + 1) * CH],
                    start=True, stop=True)
                m_sb = mp.tile([128, CH], MMDT, name="m_sb", tag="m_sb")
                if (half == 1 and slot < 4) or \
                        mcopy_i % M_COPY_ACT_EVERY == 0:
                    nc.scalar.activation(m_sb, m_ps, AF.Relu, bias=bm_sb,
                                         scale=1.0)
                else:
                    nc.vector.tensor_scalar(m_sb, m_ps, bm_sb, 0.0,
                                            ALU.add, ALU.max)
                mcopy_i += 1

                nc.tensor.matmul(
                    head_ps[half],
                    wz_sb[:, 31 - jh:159 - jh],
                    m_sb,
                    start=(step == 0), stop=(step == 31),
                )

            if half == 0 and slot == 3:
                # traced here so the engines' in-order streams put the
                # first slots' PSUM-draining copies ahead of the QP math
                qp_pre[0] = _qp_pre(nc, qp, 0, x_half[0], std4, mean4)
                qp_pre[1] = _qp_pre(nc, qp, 1, x_half[1], std4, mean4)

        # drain this half to QP while the other half's matmuls run
        hsb = hs.tile([128, CH], F32, name=f"hsb{half}", tag=f"hsb{half}")
        nc.scalar.activation(hsb[0:64, :], head_ps[half][0:64, :],
                             AF.Identity, bias=bh_sb[0:64, :], scale=-1.0)
        nc.scalar.activation(hsb[64:128, :], head_ps[half][64:128, :],
                             AF.Sigmoid, bias=bh_sb[64:128, :], scale=1.0)
        _qp_post(nc, qp, half, qp_pre[half], hsb, u_out, nc.vector)


def _host_prepare(inputs):
    """Fold mean/std into L1, build packed weight/bias tensors."""
    x = np.ascontiguousarray(inputs["x"], dtype=np.float32)
    mean = np.asarray(inputs["mean"], dtype=np.float32)
    std = np.asarray(inputs["std"], dtype=np.float32)
    W1 = np.asarray(inputs["W1"], dtype=np.float32)
    b1 = np.asarray(inputs["b1"], dtype=np.float32)
    W21 = np.asarray(inputs["W21"], dtype=np.float32)
    b21 = np.asarray(inputs["b21"], dtype=np.float32)
    W22 = np.asarray(inputs["W22"], dtype=np.float32)
    b22 = np.asarray(inputs["b22"], dtype=np.float32)
    W31 = np.asarray(inputs["W31"], dtype=np.float32)
    b31 = np.asarray(inputs["b31"], dtype=np.float32)
    W32 = np.asarray(inputs["W32"], dtype=np.float32)
    b32 = np.asarray(inputs["b32"], dtype=np.float32)

    W1eff = W1 * std[None, :]                      # [128, 8]
    b1eff = (b1 + W1 @ mean).astype(np.float32)    # [128]
    w1t = np.ascontiguousarray(W1eff.T)            # [8, 128]

    Wmid = np.vstack([W21, W22]).astype(np.float32)   # [128, 128]
    wmt = np.ascontiguousarray(Wmid.T)
    bmid = np.concatenate([b21, b22]).astype(np.float32)[:, None]

    Whead = np.zeros((4, 128), np.float32)
    Whead[0:2, 0:64] = W31
    Whead[2:4, 64:128] = W32
    wz = np.zeros((128, 160), np.float32)
    for v in range(4):
        wz[:, 31 + 32 * v] = Whead[v, :]

    bhead = np.zeros((128, 1), np.float32)
    bhead[0:32, 0] = -b31[0]
    bhead[32:64, 0] = -b31[1]
    bhead[64:96, 0] = b32[0]
    bhead[96:128, 0] = b32[1]

    std4 = tuple(float(std[i]) for i in range(4))
    mean4 = tuple(float(mean[i]) for i in range(4))

    if MMNP is None:
        import ml_dtypes
        mmnp = ml_dtypes.bfloat16
    else:
        mmnp = MMNP
    w1t = w1t.astype(mmnp)
    wmt = wmt.astype(mmnp)
    wz = wz.astype(mmnp)

    wmw = np.ascontiguousarray(np.concatenate([wmt, wz], axis=1))
    bias3 = np.ascontiguousarray(
        np.concatenate([b1eff[:, None], bmid, bhead], axis=1))

    common = {
        "w1t": w1t,
        "wmw": wmw,
        "bias3": bias3,
    }

    in_maps = []
    for c in range(NCORES):
        xs = x[c * BC:(c + 1) * BC]               # [32768, 8]
        # transposed / grouped layout: row 8g+f = feature f of group g
        xtr = np.ascontiguousarray(
            xs.reshape(NGRP, GSZ, NF).transpose(0, 2, 1).reshape(
                32, GSZ)).astype(mmnp)
        in_maps.append({"x_bm": xs, "x_tr": xtr, **common})
    return in_maps, std4, mean4


def kernel(**inputs):
    in_maps, std4, mean4 = _host_prepare(inputs)
    nc = _build_program(std4, mean4)
    last_err = None
    for attempt in range(3):
        try:
            res = run_bass_kernel_spmd(nc, in_maps, list(range(NCORES)))
            break
        except Exception as e:  # transient axon/NRT flakes
            last_err = e
            if attempt == 2:
                raise
            import time

            time.sleep(5)
    u = np.concatenate([res.results[c]["u"] for c in range(NCORES)], axis=0)
    return u.astype(np.float32)


if __name__ == "__main__":
    rng = np.random.default_rng(0)
    demo = {
        "x": rng.standard_normal((B, NF), dtype=np.float32),
        "mean": np.zeros(NF, np.float32),
        "std": np.ones(NF, np.float32),
        "W1": rng.standard_normal((128, NF), dtype=np.float32) * 0.3,
        "b1": rng.standard_normal(128, dtype=np.float32) * 0.3,
        "W21": rng.standard_normal((64, 128), dtype=np.float32) * 0.08,
        "b21": rng.standard_normal(64, dtype=np.float32) * 0.08,
        "W22": rng.standard_normal((64, 128), dtype=np.float32) * 0.08,
        "b22": rng.standard_normal(64, dtype=np.float32) * 0.08,
        "W31": rng.standard_normal((2, 64), dtype=np.float32) * 0.1,
        "b31": rng.standard_normal(2, dtype=np.float32) * 0.1,
        "W32": rng.standard_normal((2, 64), dtype=np.float32) * 0.1,
        "b32": rng.standard_normal(2, dtype=np.float32) * 0.1,
        "sgn": np.int64(1),
    }
    out = kernel(**demo)
    print(out.shape, out.dtype)

